# revision 3
# baseline (speedup 1.0000x reference)
"""MultiHuberLoss Trainium2 kernel (v2).

Reference (per element, with m = +x at the target class, -x elsewhere):
    hinge = max(0, 1 - m);  loss = where(m >= -1, hinge^2, -4m);  out = sum(loss)/N

Identities (exact):
  F(m) = relu(1-m)^2 - relu(-1-m)^2
  Main pass uses m = -x for EVERY element:
      F(-x) = (clamp(x,-1,1)+1)^2 + 4*relu(x-1)
  Per-row correction for the target column t (where m = +x_t):
      F(x_t) - F(-x_t) = -4 * x_t

Engine split (per core, 16 tiles of [128, 4000] fp32):
  - DVE:   c = clamp(x,-1,1) -> fp16            (tensor_scalar, 2x_2P mode)
           w = 4*max(x,1)    -> fp16            (tensor_scalar, 2x_2P mode;
                                                 12 non-STT tiles)
  - ACT:   Square(c+1) with accum_out -> accQ   (all 16 tiles)
           Relu(x-1)  with accum_out -> accB    (4 STT tiles only)
  - PE:    per-chunk matmul(ones, w) accumulated into one PSUM bank --
           replaces the 1x-mode DVE accumulate pass of the baseline
  - target extraction: 48 row-groups via gpsimd indirect-DMA gathers,
           16 row-groups via DVE is_equal masks (STT tiles)
  - finale: entirely on PE (partition-sums of G/gstt/accQ/accB into PSUM)
           + ACT (PSUM reads, combine, scale); the DVE stream ends with the
           main loop, so the gather chain never head-of-line blocks it.
           ACT finale instructions write their (discarded) elementwise
           outputs into the act_scr scratch so a same-engine WAW pins them
           after the whole main ACT stream.
"""

import numpy as np

import concourse.bacc as bacc
import concourse.bass as bass
import concourse.mybir as mybir
from concourse.bass_utils import run_bass_kernel_spmd
from concourse.tile import TileContext

N_TOTAL = 65536
C = 1000
N_CORES = 8
ROWS = N_TOTAL // N_CORES  # 8192 rows per core
P = 128                    # partitions
JPP = ROWS // P            # 64 rows per partition
FREE = JPP * C             # 64000 f32 per partition
FD = 4000                  # free-dim tile size (4 whole rows per partition)
NT = FREE // FD            # 16 tiles
RPT = FD // C              # rows per partition per tile (4)
CH = 500                   # PE chunk width (fits one PSUM bank)

# tiles whose sub-rows get their target extracted via DVE is_equal-mask;
# all other tiles compute their B-term (4*sum max(x,1)) on DVE+PE
STT_TILES = (2, 5, 8, 11)

f32 = mybir.dt.float32
f16 = mybir.dt.float16
i32 = mybir.dt.int32
Alu = mybir.AluOpType
Act = mybir.ActivationFunctionType


def build_program():
    nc = bacc.Bacc(
        "TRN2", target_bir_lowering=False, debug=False, num_devices=N_CORES
    )
    x = nc.dram_tensor("x", [ROWS, C], f32, kind="ExternalInput")
    # host-precomputed flat element offsets: og[r] = r*C + target[r]
    og = nc.dram_tensor("og", [ROWS], i32, kind="ExternalInput")
    # target column of each row as f32 (for the is_equal extraction)
    tc_in = nc.dram_tensor("tc", [ROWS], f32, kind="ExternalInput")
    out = nc.dram_tensor("out", [1, 1], f32, kind="ExternalOutput")

    x_flat = x.ap().rearrange("(p j) c -> p (j c)", p=P)  # [128, 64000]
    x_lin = x.ap().rearrange("a (b one) -> (a b) one", one=1)  # [8192000, 1]
    og2d = og.ap().rearrange("(p j) -> p j", p=P)         # [128, 64]
    tc2d = tc_in.ap().rearrange("(p j) -> p j", p=P)      # [128, 64]

    stt_js = {t * RPT + j for t in STT_TILES for j in range(RPT)}
    n_stt = len(stt_js)
    n_dveb = NT - len(STT_TILES)

    with TileContext(nc) as tc:
        with (
            tc.tile_pool(name="xp", bufs=4) as xp,
            tc.tile_pool(name="cp", bufs=3) as cp,
            tc.tile_pool(name="wp", bufs=3) as wp,
            tc.tile_pool(name="scr", bufs=1) as scr,
            tc.tile_pool(name="small", bufs=1) as small,
            tc.tile_pool(name="psp", bufs=1, space="PSUM") as psp,
        ):
            ones_h = small.tile([P, 1], f16, tag="ones_h")
            nc.vector.memset(ones_h[:], 1.0)
            ones_f = small.tile([P, 1], f32, tag="ones_f")
            nc.vector.memset(ones_f[:], 1.0)
            negones = small.tile([P, 1], f32, tag="negones")
            nc.vector.memset(negones[:], -1.0)
            # column-index ramp 0..999, same on every partition (f32 exact)
            ci = small.tile([P, C], f32, tag="ci")
            nc.gpsimd.iota(
                ci[:], pattern=[[1, C]], base=0, channel_multiplier=0,
                allow_small_or_imprecise_dtypes=True,
            )

            # ---- gather path, traced FIRST so the offsets DMA leads the
            # Sync queue and the gathers start on gpsimd early ----
            offs = small.tile([P, JPP], i32, tag="offs")
            nc.sync.dma_start(out=offs[:], in_=og2d)
            tcv = small.tile([P, JPP], f32, tag="tcv")
            nc.sync.dma_start(out=tcv[:], in_=tc2d)
            G = small.tile([P, JPP], f32, tag="G")
            nc.vector.memset(G[:], 0.0)
            for j in range(JPP):
                if j in stt_js:
                    continue
                nc.gpsimd.indirect_dma_start(
                    out=G[:, j:j + 1],
                    out_offset=None,
                    in_=x_lin,
                    in_offset=bass.IndirectOffsetOnAxis(
                        ap=offs[:, j:j + 1], axis=0
                    ),
                )

            # ---- main streaming loop ----
            accQ = small.tile([P, NT], f32, tag="accQ")
            accB = small.tile([P, len(STT_TILES)], f32, tag="accB")
            gstt = small.tile([P, max(1, n_stt)], f32, tag="gstt")
            psA = psp.tile([1, CH], f32, tag="psA")
            psB = psp.tile([1, 64 + n_stt + NT + len(STT_TILES)], f32,
                           tag="psB")
            si = 0
            bi = 0
            n_chunks = n_dveb * (FD // CH)
            ck = 0
            for t in range(NT):
                xt = xp.tile([P, FD], f32)
                # two half-tile DMAs: finer SDMA descriptors (8KB) so the
                # concurrent gather descriptors aren't stuck behind big
                # blocking quanta on the shared SDMA engines
                h = FD // 2
                nc.sync.dma_start(
                    out=xt[:, 0:h], in_=x_flat[:, t * FD:t * FD + h]
                )
                nc.sync.dma_start(
                    out=xt[:, h:FD], in_=x_flat[:, t * FD + h:(t + 1) * FD]
                )
                c = cp.tile([P, FD], f16)
                nc.vector.tensor_scalar(
                    c[:], xt[:], -1.0, 1.0, Alu.max, Alu.min
                )
                # accQ col = sum (c+1)^2
                sq = scr.tile([P, FD], f16, tag="act_scr")
                nc.scalar.activation(
                    sq[:],
                    c[:],
                    Act.Square,
                    bias=1.0,
                    scale=1.0,
                    accum_out=accQ[:, t:t + 1],
                )
                if t in STT_TILES:
                    # accB col = sum relu(x-1)  (B-term on ACT for this tile)
                    rl = scr.tile([P, FD], f16, tag="act_scr")
                    nc.scalar.activation(
                        rl[:],
                        xt[:],
                        Act.Relu,
                        bias=negones[:],
                        scale=1.0,
                        accum_out=accB[:, bi:bi + 1],
                    )
                    bi += 1
                    # per-sub-row target extraction:
                    #   gstt col = sum( (ci == target_col) * x_subrow )
                    for j in range(RPT):
                        ws = scr.tile([P, C], f32, tag="w_stt")
                        nc.vector.scalar_tensor_tensor(
                            out=ws[:],
                            in0=ci[:],
                            scalar=tcv[:, t * RPT + j:t * RPT + j + 1],
                            in1=xt[:, j * C:(j + 1) * C],
                            op0=Alu.is_equal,
                            op1=Alu.mult,
                            accum_out=gstt[:, si:si + 1],
                        )
                        si += 1
                else:
                    # w = 4*max(x,1) (fp16), partition-summed on PE into psA
                    w = wp.tile([P, FD], f16)
                    nc.vector.tensor_scalar(
                        w[:], xt[:], 1.0, 4.0, Alu.max, Alu.mult
                    )
                    for k in range(FD // CH):
                        nc.tensor.matmul(
                            out=psA[:],
                            lhsT=ones_h[:],
                            rhs=w[:, k * CH:(k + 1) * CH],
                            start=(ck == 0),
                            stop=(ck == n_chunks - 1),
                        )
                        ck += 1

            # ---- finale: PE partition-sums, then ACT combine ----
            nc.tensor.matmul(
                out=psB[:, 0:JPP], lhsT=ones_f[:], rhs=G[:],
                start=True, stop=True,
            )
            o1 = JPP
            nc.tensor.matmul(
                out=psB[:, o1:o1 + n_stt], lhsT=ones_f[:], rhs=gstt[:],
                start=True, stop=True,
            )
            o2 = o1 + n_stt
            nc.tensor.matmul(
                out=psB[:, o2:o2 + NT], lhsT=ones_f[:], rhs=accQ[:],
                start=True, stop=True,
            )
            o3 = o2 + NT
            nc.tensor.matmul(
                out=psB[:, o3:o3 + len(STT_TILES)], lhsT=ones_f[:],
                rhs=accB[:], start=True, stop=True,
            )
            o4 = o3 + len(STT_TILES)

            # ACT combine; elementwise outputs land in act_scr so a
            # same-engine WAW keeps these after the whole main ACT stream
            fin = small.tile([1, 8], f32, tag="fin")
            pin = scr.tile([P, FD], f16, tag="act_scr")
            # fA = -4 * (sum G + sum gstt)   (the -4*x_t correction)
            nc.scalar.activation(
                pin[0:1, 0:o2], psB[:, 0:o2], Act.Identity,
                bias=0.0, scale=-4.0, accum_out=fin[:, 0:1],
            )
            # fB = sum (c+1)^2 over all tiles
            nc.scalar.activation(
                pin[0:1, 0:NT], psB[:, o2:o3], Act.Identity,
                bias=0.0, scale=1.0, accum_out=fin[:, 1:2],
            )
            # fC = 4 * sum relu(x-1) over STT tiles
            nc.scalar.activation(
                pin[0:1, 0:len(STT_TILES)], psB[:, o3:o4], Act.Identity,
                bias=0.0, scale=4.0, accum_out=fin[:, 2:3],
            )
            # fD = sum 4*max(x,1) over the 12 DVE-B tiles
            nc.scalar.activation(
                pin[0:1, 0:CH], psA[:], Act.Identity,
                bias=0.0, scale=1.0, accum_out=fin[:, 3:4],
            )
            g1 = small.tile([1, 1], f32, tag="g1")
            nc.scalar.activation(
                g1[:], fin[:, 0:1], Act.Identity, bias=fin[:, 1:2], scale=1.0
            )
            g2 = small.tile([1, 1], f32, tag="g2")
            nc.scalar.activation(
                g2[:], fin[:, 2:3], Act.Identity, bias=fin[:, 3:4], scale=1.0
            )
            g3 = small.tile([1, 1], f32, tag="g3")
            nc.scalar.activation(
                g3[:], g1[:], Act.Identity, bias=g2[:], scale=1.0
            )
            # res = g3/N + const/N, const = -4*FD*P per DVE-B tile
            # (w = 4*max(x,1) = 4*relu(x-1) + 4 per element)
            biasc = (-4.0 * FD * P * n_dveb) / N_TOTAL
            bias_t = small.tile([1, 1], f32, tag="bias")
            nc.vector.memset(bias_t[:], biasc)
            res = small.tile([1, 1], f32, tag="res")
            nc.scalar.activation(
                res[:], g3[:], Act.Identity, bias=bias_t[:],
                scale=1.0 / N_TOTAL,
            )
            nc.sync.dma_start(out=out.ap(), in_=res[:])

    nc.compile()
    return nc


_NC_CACHE = None
LAST_RESULTS = None


def kernel(input, target):
    global _NC_CACHE, LAST_RESULTS
    x = np.ascontiguousarray(np.asarray(input, dtype=np.float32))
    tg = np.ascontiguousarray(np.asarray(target).astype(np.int64))
    assert x.shape == (N_TOTAL, C), x.shape
    assert tg.shape == (N_TOTAL,), tg.shape

    if _NC_CACHE is None:
        _NC_CACHE = build_program()
    nc = _NC_CACHE

    # flat element offset of each row's target within its core shard
    offs_all = (
        np.tile(np.arange(ROWS, dtype=np.int64) * C, N_CORES) + tg
    ).astype(np.int32)
    tc_all = tg.astype(np.float32)

    in_maps = [
        {
            "x": x[c * ROWS:(c + 1) * ROWS],
            "og": offs_all[c * ROWS:(c + 1) * ROWS],
            "tc": tc_all[c * ROWS:(c + 1) * ROWS],
        }
        for c in range(N_CORES)
    ]
    res = run_bass_kernel_spmd(nc, in_maps, core_ids=list(range(N_CORES)))
    LAST_RESULTS = res
    total = np.float32(0.0)
    for r in res.results:
        total += np.float32(r["out"].reshape(()))
    return np.asarray(total, dtype=np.float32)


if __name__ == "__main__":
    rng = np.random.default_rng(0)
    xs = rng.standard_normal((N_TOTAL, C), dtype=np.float32)
    ts = rng.integers(0, C, size=(N_TOTAL,)).astype(np.int64)
    got = kernel(xs, ts)
    m = np.where(np.arange(C)[None, :] == ts[:, None], xs, -xs)
    hinge = np.maximum(0.0, 1.0 - m)
    loss = np.where(m >= -1.0, hinge * hinge, -4.0 * m)
    want = loss.sum(dtype=np.float64) / N_TOTAL
    print("got", got, "want", want, "rel", abs(got - want) / abs(want))


# revision 6
# speedup vs baseline: 1.1846x; 1.1846x over previous
"""MultiHuberLoss Trainium2 kernel (v2).

Reference (per element, with m = +x at the target class, -x elsewhere):
    hinge = max(0, 1 - m);  loss = where(m >= -1, hinge^2, -4m);  out = sum(loss)/N

Identities (exact):
  F(m) = relu(1-m)^2 - relu(-1-m)^2
  Main pass uses m = -x for EVERY element:
      F(-x) = (clamp(x,-1,1)+1)^2 + 4*relu(x-1)
  Per-row correction for the target column t (where m = +x_t):
      F(x_t) - F(-x_t) = -4 * x_t

Engine split (per core, 16 tiles of [128, 4000] fp32):
  - DVE:   c = clamp(x,-1,1) -> fp16            (tensor_scalar, 2x_2P mode)
           w = 4*max(x,1)    -> fp16            (tensor_scalar, 2x_2P mode;
                                                 12 non-STT tiles)
  - ACT:   Square(c+1) with accum_out -> accQ   (all 16 tiles)
           Relu(x-1)  with accum_out -> accB    (4 STT tiles only)
  - PE:    per-chunk matmul(ones, w) accumulated into one PSUM bank --
           replaces the 1x-mode DVE accumulate pass of the baseline
  - target extraction: 48 row-groups via gpsimd indirect-DMA gathers,
           16 row-groups via DVE is_equal masks (STT tiles)
  - finale: entirely on PE (partition-sums of G/gstt/accQ/accB into PSUM)
           + ACT (PSUM reads, combine, scale); the DVE stream ends with the
           main loop, so the gather chain never head-of-line blocks it.
           ACT finale instructions write their (discarded) elementwise
           outputs into the act_scr scratch so a same-engine WAW pins them
           after the whole main ACT stream.
"""

import numpy as np

import concourse.bacc as bacc
import concourse.bass as bass
import concourse.mybir as mybir
from concourse.bass_utils import run_bass_kernel_spmd
from concourse.tile import TileContext

N_TOTAL = 65536
C = 1000
N_CORES = 8
ROWS = N_TOTAL // N_CORES  # 8192 rows per core
P = 128                    # partitions
JPP = ROWS // P            # 64 rows per partition
FREE = JPP * C             # 64000 f32 per partition
FD = 4000                  # free-dim tile size (4 whole rows per partition)
NT = FREE // FD            # 16 tiles
RPT = FD // C              # rows per partition per tile (4)
CH = 500                   # PE chunk width (fits one PSUM bank)

# tiles whose sub-rows get their target extracted via DVE is_equal-mask;
# all other tiles compute their B-term (4*sum max(x,1)) on DVE+PE
STT_TILES = (2, 5, 8, 11)

f32 = mybir.dt.float32
f16 = mybir.dt.float16
i32 = mybir.dt.int32
Alu = mybir.AluOpType
Act = mybir.ActivationFunctionType


def build_program():
    nc = bacc.Bacc(
        "TRN2", target_bir_lowering=False, debug=False, num_devices=N_CORES
    )
    x = nc.dram_tensor("x", [ROWS, C], f32, kind="ExternalInput")
    # host-precomputed flat element offsets: og[r] = r*C + target[r]
    og = nc.dram_tensor("og", [ROWS], i32, kind="ExternalInput")
    # target column of each row as f32 (for the is_equal extraction)
    tc_in = nc.dram_tensor("tc", [ROWS], f32, kind="ExternalInput")
    out = nc.dram_tensor("out", [1, 1], f32, kind="ExternalOutput")

    x_flat = x.ap().rearrange("(p j) c -> p (j c)", p=P)  # [128, 64000]
    x_lin = x.ap().rearrange("a (b one) -> (a b) one", one=1)  # [8192000, 1]
    og2d = og.ap().rearrange("(p j) -> p j", p=P)         # [128, 64]
    tc2d = tc_in.ap().rearrange("(p j) -> p j", p=P)      # [128, 64]

    stt_js = {t * RPT + j for t in STT_TILES for j in range(RPT)}
    n_stt = len(stt_js)
    n_dveb = NT - len(STT_TILES)

    with TileContext(nc) as tc:
        with (
            tc.tile_pool(name="xp", bufs=5) as xp,
            tc.tile_pool(name="cp", bufs=4) as cp,
            tc.tile_pool(name="wp", bufs=4) as wp,
            tc.tile_pool(name="scr", bufs=1) as scr,
            tc.tile_pool(name="small", bufs=1) as small,
            tc.tile_pool(name="psp", bufs=1, space="PSUM") as psp,
        ):
            ones_h = small.tile([P, 1], f16, tag="ones_h")
            nc.vector.memset(ones_h[:], 1.0)
            ones_f = small.tile([P, 1], f32, tag="ones_f")
            nc.vector.memset(ones_f[:], 1.0)
            negones = small.tile([P, 1], f32, tag="negones")
            nc.vector.memset(negones[:], -1.0)
            # column-index ramp 0..999, same on every partition (f32 exact)
            ci = small.tile([P, C], f32, tag="ci")
            nc.gpsimd.iota(
                ci[:], pattern=[[1, C]], base=0, channel_multiplier=0,
                allow_small_or_imprecise_dtypes=True,
            )

            # ---- gather path, traced FIRST so the offsets DMA leads the
            # Sync queue and the gathers start on gpsimd early ----
            offs = small.tile([P, JPP], i32, tag="offs")
            nc.sync.dma_start(out=offs[:], in_=og2d)
            tcv = small.tile([P, JPP], f32, tag="tcv")
            nc.sync.dma_start(out=tcv[:], in_=tc2d)
            G = small.tile([P, JPP], f32, tag="G")
            nc.vector.memset(G[:], 0.0)
            for j in range(JPP):
                if j in stt_js:
                    continue
                nc.gpsimd.indirect_dma_start(
                    out=G[:, j:j + 1],
                    out_offset=None,
                    in_=x_lin,
                    in_offset=bass.IndirectOffsetOnAxis(
                        ap=offs[:, j:j + 1], axis=0
                    ),
                )

            # ---- main streaming loop ----
            accQ = small.tile([P, NT], f32, tag="accQ")
            accB = small.tile([P, len(STT_TILES)], f32, tag="accB")
            gstt = small.tile([P, max(1, n_stt)], f32, tag="gstt")
            psA = psp.tile([1, CH], f32, tag="psA")
            si = 0
            bi = 0
            n_chunks = n_dveb * (FD // CH)
            ck = 0
            for t in range(NT):
                xt = xp.tile([P, FD], f32)
                # two half-tile DMAs: finer SDMA descriptors (8KB) so the
                # concurrent gather descriptors aren't stuck behind big
                # blocking quanta on the shared SDMA engines
                h = FD // 2
                nc.sync.dma_start(
                    out=xt[:, 0:h], in_=x_flat[:, t * FD:t * FD + h]
                )
                nc.sync.dma_start(
                    out=xt[:, h:FD], in_=x_flat[:, t * FD + h:(t + 1) * FD]
                )
                c = cp.tile([P, FD], f16)
                nc.vector.tensor_scalar(
                    c[:], xt[:], -1.0, 1.0, Alu.max, Alu.min
                )
                # accQ col = sum (c+1)^2
                sq = scr.tile([P, FD], f16, tag="act_scr")
                nc.scalar.activation(
                    sq[:],
                    c[:],
                    Act.Square,
                    bias=1.0,
                    scale=1.0,
                    accum_out=accQ[:, t:t + 1],
                )
                if t in STT_TILES:
                    # accB col = sum relu(x-1)  (B-term on ACT for this tile)
                    rl = scr.tile([P, FD], f16, tag="act_scr")
                    nc.scalar.activation(
                        rl[:],
                        xt[:],
                        Act.Relu,
                        bias=negones[:],
                        scale=1.0,
                        accum_out=accB[:, bi:bi + 1],
                    )
                    bi += 1
                    # per-sub-row target extraction:
                    #   gstt col = sum( (ci == target_col) * x_subrow )
                    for j in range(RPT):
                        ws = scr.tile([P, C], f32, tag="w_stt")
                        nc.vector.scalar_tensor_tensor(
                            out=ws[:],
                            in0=ci[:],
                            scalar=tcv[:, t * RPT + j:t * RPT + j + 1],
                            in1=xt[:, j * C:(j + 1) * C],
                            op0=Alu.is_equal,
                            op1=Alu.mult,
                            accum_out=gstt[:, si:si + 1],
                        )
                        si += 1
                else:
                    # w = 4*max(x,1) (fp16), partition-summed on PE into psA
                    w = wp.tile([P, FD], f16)
                    nc.vector.tensor_scalar(
                        w[:], xt[:], 1.0, 4.0, Alu.max, Alu.mult
                    )
                    for k in range(FD // CH):
                        nc.tensor.matmul(
                            out=psA[:],
                            lhsT=ones_h[:],
                            rhs=w[:, k * CH:(k + 1) * CH],
                            start=(ck == 0),
                            stop=(ck == n_chunks - 1),
                        )
                        ck += 1

            # ---- finale ----
            # fD = sum 4*max(x,1) over the 12 DVE-B tiles, read out FIRST
            # (the PE partition-sums below clobber psA columns; reusing the
            # psA tile gives the finale matmuls a WAW dep on every chunk
            # matmul, so the scheduler cannot hoist them ahead of the main
            # loop and head-of-line block the PE queue on the gather chain)
            fin = small.tile([1, 8], f32, tag="fin")
            pin = scr.tile([P, FD], f16, tag="act_scr")
            nc.scalar.activation(
                pin[0:1, 0:CH], psA[:], Act.Identity,
                bias=0.0, scale=1.0, accum_out=fin[:, 3:4],
            )
            # PE partition-sums of the small accumulators into psA columns
            nc.tensor.matmul(
                out=psA[:, 0:JPP], lhsT=ones_f[:], rhs=G[:],
                start=True, stop=True,
            )
            o1 = JPP
            nc.tensor.matmul(
                out=psA[:, o1:o1 + n_stt], lhsT=ones_f[:], rhs=gstt[:],
                start=True, stop=True,
            )
            o2 = o1 + n_stt
            nc.tensor.matmul(
                out=psA[:, o2:o2 + NT], lhsT=ones_f[:], rhs=accQ[:],
                start=True, stop=True,
            )
            o3 = o2 + NT
            nc.tensor.matmul(
                out=psA[:, o3:o3 + len(STT_TILES)], lhsT=ones_f[:],
                rhs=accB[:], start=True, stop=True,
            )
            o4 = o3 + len(STT_TILES)

            # ACT combine; elementwise outputs land in act_scr so a
            # same-engine WAW keeps these after the whole main ACT stream
            # fA = -4 * (sum G + sum gstt)   (the -4*x_t correction)
            nc.scalar.activation(
                pin[0:1, 0:o2], psA[:, 0:o2], Act.Identity,
                bias=0.0, scale=-4.0, accum_out=fin[:, 0:1],
            )
            # fB = sum (c+1)^2 over all tiles
            nc.scalar.activation(
                pin[0:1, 0:NT], psA[:, o2:o3], Act.Identity,
                bias=0.0, scale=1.0, accum_out=fin[:, 1:2],
            )
            # fC = 4 * sum relu(x-1) over STT tiles
            nc.scalar.activation(
                pin[0:1, 0:len(STT_TILES)], psA[:, o3:o4], Act.Identity,
                bias=0.0, scale=4.0, accum_out=fin[:, 2:3],
            )
            g1 = small.tile([1, 1], f32, tag="g1")
            nc.scalar.activation(
                g1[:], fin[:, 0:1], Act.Identity, bias=fin[:, 1:2], scale=1.0
            )
            g2 = small.tile([1, 1], f32, tag="g2")
            nc.scalar.activation(
                g2[:], fin[:, 2:3], Act.Identity, bias=fin[:, 3:4], scale=1.0
            )
            g3 = small.tile([1, 1], f32, tag="g3")
            nc.scalar.activation(
                g3[:], g1[:], Act.Identity, bias=g2[:], scale=1.0
            )
            # res = g3/N + const/N, const = -4*FD*P per DVE-B tile
            # (w = 4*max(x,1) = 4*relu(x-1) + 4 per element)
            biasc = (-4.0 * FD * P * n_dveb) / N_TOTAL
            bias_t = small.tile([1, 1], f32, tag="bias")
            nc.vector.memset(bias_t[:], biasc)
            res = small.tile([1, 1], f32, tag="res")
            nc.scalar.activation(
                res[:], g3[:], Act.Identity, bias=bias_t[:],
                scale=1.0 / N_TOTAL,
            )
            nc.sync.dma_start(out=out.ap(), in_=res[:])

    nc.compile()
    return nc


_NC_CACHE = None
LAST_RESULTS = None


def kernel(input, target):
    global _NC_CACHE, LAST_RESULTS
    x = np.ascontiguousarray(np.asarray(input, dtype=np.float32))
    tg = np.ascontiguousarray(np.asarray(target).astype(np.int64))
    assert x.shape == (N_TOTAL, C), x.shape
    assert tg.shape == (N_TOTAL,), tg.shape

    if _NC_CACHE is None:
        _NC_CACHE = build_program()
    nc = _NC_CACHE

    # flat element offset of each row's target within its core shard
    offs_all = (
        np.tile(np.arange(ROWS, dtype=np.int64) * C, N_CORES) + tg
    ).astype(np.int32)
    tc_all = tg.astype(np.float32)

    in_maps = [
        {
            "x": x[c * ROWS:(c + 1) * ROWS],
            "og": offs_all[c * ROWS:(c + 1) * ROWS],
            "tc": tc_all[c * ROWS:(c + 1) * ROWS],
        }
        for c in range(N_CORES)
    ]
    res = run_bass_kernel_spmd(nc, in_maps, core_ids=list(range(N_CORES)))
    LAST_RESULTS = res
    total = np.float32(0.0)
    for r in res.results:
        total += np.float32(r["out"].reshape(()))
    return np.asarray(total, dtype=np.float32)


if __name__ == "__main__":
    rng = np.random.default_rng(0)
    xs = rng.standard_normal((N_TOTAL, C), dtype=np.float32)
    ts = rng.integers(0, C, size=(N_TOTAL,)).astype(np.int64)
    got = kernel(xs, ts)
    m = np.where(np.arange(C)[None, :] == ts[:, None], xs, -xs)
    hinge = np.maximum(0.0, 1.0 - m)
    loss = np.where(m >= -1.0, hinge * hinge, -4.0 * m)
    want = loss.sum(dtype=np.float64) / N_TOTAL
    print("got", got, "want", want, "rel", abs(got - want) / abs(want))


# revision 12
# speedup vs baseline: 1.2712x; 1.0732x over previous
"""MultiHuberLoss Trainium2 kernel (v2).

Reference (per element, with m = +x at the target class, -x elsewhere):
    hinge = max(0, 1 - m);  loss = where(m >= -1, hinge^2, -4m);  out = sum(loss)/N

Identities (exact):
  F(m) = relu(1-m)^2 - relu(-1-m)^2
  Main pass uses m = -x for EVERY element:
      F(-x) = (clamp(x,-1,1)+1)^2 + 4*relu(x-1)
  Per-row correction for the target column t (where m = +x_t):
      F(x_t) - F(-x_t) = -4 * x_t

Engine split (per core, 16 tiles of [128, 4000] fp32):
  - DVE:   c = clamp(x,-1,1) -> fp16            (tensor_scalar, 2x_2P mode)
           w = 4*max(x,1)    -> fp16            (tensor_scalar, 2x_2P mode;
                                                 12 non-STT tiles)
  - ACT:   Square(c+1) with accum_out -> accQ   (all 16 tiles)
           Relu(x-1)  with accum_out -> accB    (4 STT tiles only)
  - PE:    per-chunk matmul(ones, w) accumulated into one PSUM bank --
           replaces the 1x-mode DVE accumulate pass of the baseline
  - target extraction: 48 row-groups via gpsimd indirect-DMA gathers,
           16 row-groups via DVE is_equal masks (STT tiles)
  - finale: entirely on PE (partition-sums of G/gstt/accQ/accB into PSUM)
           + ACT (PSUM reads, combine, scale); the DVE stream ends with the
           main loop, so the gather chain never head-of-line blocks it.
           ACT finale instructions write their (discarded) elementwise
           outputs into the act_scr scratch so a same-engine WAW pins them
           after the whole main ACT stream.
"""

import numpy as np

import concourse.bacc as bacc
import concourse.bass as bass
import concourse.mybir as mybir
from concourse.bass_utils import run_bass_kernel_spmd
from concourse.tile import TileContext

N_TOTAL = 65536
C = 1000
N_CORES = 8
ROWS = N_TOTAL // N_CORES  # 8192 rows per core
P = 128                    # partitions
JPP = ROWS // P            # 64 rows per partition
FREE = JPP * C             # 64000 f32 per partition
FD = 4000                  # free-dim tile size (4 whole rows per partition)
NT = FREE // FD            # 16 tiles
RPT = FD // C              # rows per partition per tile (4)
CH = 500                   # PE chunk width (fits one PSUM bank)

# tiles whose sub-rows get their target extracted via DVE is_equal-mask
# (28 row-groups; the other 36 row-groups use gpsimd indirect gathers)
STT_TILES = (1, 3, 5, 7, 9, 11, 13)
# tiles whose B-term (4*sum max(x,1)) runs on DVE+PE; the rest use ACT Relu
DVEB_TILES = (0, 2, 4, 6, 8, 10, 12, 13, 14, 15)

f32 = mybir.dt.float32
f16 = mybir.dt.float16
i32 = mybir.dt.int32
Alu = mybir.AluOpType
Act = mybir.ActivationFunctionType


def build_program():
    nc = bacc.Bacc(
        "TRN2", target_bir_lowering=False, debug=False, num_devices=N_CORES
    )
    x = nc.dram_tensor("x", [ROWS, C], f32, kind="ExternalInput")
    # host-precomputed flat element offsets: og[r] = r*C + target[r]
    og = nc.dram_tensor("og", [ROWS], i32, kind="ExternalInput")
    # target column of each row as f32 (for the is_equal extraction)
    tc_in = nc.dram_tensor("tc", [ROWS], f32, kind="ExternalInput")
    out = nc.dram_tensor("out", [1, 1], f32, kind="ExternalOutput")

    x_flat = x.ap().rearrange("(p j) c -> p (j c)", p=P)  # [128, 64000]
    x_lin = x.ap().rearrange("a (b one) -> (a b) one", one=1)  # [8192000, 1]
    og2d = og.ap().rearrange("(p j) -> p j", p=P)         # [128, 64]
    tc2d = tc_in.ap().rearrange("(p j) -> p j", p=P)      # [128, 64]

    stt_js = {t * RPT + j for t in STT_TILES for j in range(RPT)}
    n_stt = len(stt_js)
    n_dveb = len(DVEB_TILES)
    n_actb = NT - n_dveb

    with TileContext(nc) as tc:
        with (
            tc.tile_pool(name="xp", bufs=5) as xp,
            tc.tile_pool(name="cp", bufs=4) as cp,
            tc.tile_pool(name="wp", bufs=4) as wp,
            tc.tile_pool(name="scr", bufs=1) as scr,
            tc.tile_pool(name="small", bufs=1) as small,
            tc.tile_pool(name="psp", bufs=1, space="PSUM") as psp,
        ):
            ones_h = small.tile([P, 1], f16, tag="ones_h")
            nc.vector.memset(ones_h[:], 1.0)
            ones_f = small.tile([P, 1], f32, tag="ones_f")
            nc.vector.memset(ones_f[:], 1.0)
            negones = small.tile([P, 1], f32, tag="negones")
            nc.vector.memset(negones[:], -1.0)
            # column-index ramp 0..999, same on every partition (f32 exact)
            ci = small.tile([P, C], f32, tag="ci")
            nc.gpsimd.iota(
                ci[:], pattern=[[1, C]], base=0, channel_multiplier=0,
                allow_small_or_imprecise_dtypes=True,
            )

            # ---- gather path, traced FIRST so the offsets DMA leads the
            # Sync queue and the gathers start on gpsimd early ----
            offs = small.tile([P, JPP], i32, tag="offs")
            nc.sync.dma_start(out=offs[:], in_=og2d)
            tcv = small.tile([P, JPP], f32, tag="tcv")
            nc.sync.dma_start(out=tcv[:], in_=tc2d)
            # memset on gpsimd: keeps the gather prologue free of
            # cross-engine waits so the indirect chain starts early
            G = small.tile([P, JPP], f32, tag="G")
            nc.gpsimd.memset(G[:], 0.0)
            for j in range(JPP):
                if j in stt_js:
                    continue
                nc.gpsimd.indirect_dma_start(
                    out=G[:, j:j + 1],
                    out_offset=None,
                    in_=x_lin,
                    in_offset=bass.IndirectOffsetOnAxis(
                        ap=offs[:, j:j + 1], axis=0
                    ),
                )

            # ---- main streaming loop ----
            accQ = small.tile([P, NT], f32, tag="accQ")
            accB = small.tile([P, n_actb], f32, tag="accB")
            gstt = small.tile([P, max(1, n_stt)], f32, tag="gstt")
            psA = psp.tile([1, CH], f32, tag="psA")
            si = 0
            bi = 0
            n_chunks = n_dveb * (FD // CH)
            ck = 0
            for t in range(NT):
                xt = xp.tile([P, FD], f32)
                # two half-tile DMAs: finer SDMA descriptors (8KB) so the
                # concurrent gather descriptors aren't stuck behind big
                # blocking quanta on the shared SDMA engines
                h = FD // 2
                nc.sync.dma_start(
                    out=xt[:, 0:h], in_=x_flat[:, t * FD:t * FD + h]
                )
                nc.sync.dma_start(
                    out=xt[:, h:FD], in_=x_flat[:, t * FD + h:(t + 1) * FD]
                )
                c = cp.tile([P, FD], f16)
                nc.vector.tensor_scalar(
                    c[:], xt[:], -1.0, 1.0, Alu.max, Alu.min
                )
                # accQ col = sum (c+1)^2
                sq = scr.tile([P, FD], f16, tag="act_scr")
                nc.scalar.activation(
                    sq[:],
                    c[:],
                    Act.Square,
                    bias=1.0,
                    scale=1.0,
                    accum_out=accQ[:, t:t + 1],
                )
                if t in DVEB_TILES:
                    # w = 4*max(x,1) (fp16), partition-summed on PE into psA
                    w = wp.tile([P, FD], f16)
                    nc.vector.tensor_scalar(
                        w[:], xt[:], 1.0, 4.0, Alu.max, Alu.mult
                    )
                    for k in range(FD // CH):
                        nc.tensor.matmul(
                            out=psA[:],
                            lhsT=ones_h[:],
                            rhs=w[:, k * CH:(k + 1) * CH],
                            start=(ck == 0),
                            stop=(ck == n_chunks - 1),
                        )
                        ck += 1
                else:
                    # accB col = sum relu(x-1)  (B-term on ACT for this tile)
                    rl = scr.tile([P, FD], f16, tag="act_scr")
                    nc.scalar.activation(
                        rl[:],
                        xt[:],
                        Act.Relu,
                        bias=negones[:],
                        scale=1.0,
                        accum_out=accB[:, bi:bi + 1],
                    )
                    bi += 1
                if t in STT_TILES:
                    # per-sub-row target extraction:
                    #   gstt col = sum( (ci == target_col) * x_subrow )
                    for j in range(RPT):
                        ws = scr.tile([P, C], f32, tag="w_stt")
                        nc.vector.scalar_tensor_tensor(
                            out=ws[:],
                            in0=ci[:],
                            scalar=tcv[:, t * RPT + j:t * RPT + j + 1],
                            in1=xt[:, j * C:(j + 1) * C],
                            op0=Alu.is_equal,
                            op1=Alu.mult,
                            accum_out=gstt[:, si:si + 1],
                        )
                        si += 1

            # ---- finale ----
            # fD = sum 4*max(x,1) over the 12 DVE-B tiles, read out FIRST
            # (the PE partition-sums below clobber psA columns; reusing the
            # psA tile gives the finale matmuls a WAW dep on every chunk
            # matmul, so the scheduler cannot hoist them ahead of the main
            # loop and head-of-line block the PE queue on the gather chain)
            fin = small.tile([1, 8], f32, tag="fin")
            pin = scr.tile([P, FD], f16, tag="act_scr")
            nc.scalar.activation(
                pin[0:1, 0:CH], psA[:], Act.Identity,
                bias=0.0, scale=1.0, accum_out=fin[:, 3:4],
            )
            # PE partition-sums of the small accumulators into psA columns
            nc.tensor.matmul(
                out=psA[:, 0:JPP], lhsT=ones_f[:], rhs=G[:],
                start=True, stop=True,
            )
            o1 = JPP
            nc.tensor.matmul(
                out=psA[:, o1:o1 + n_stt], lhsT=ones_f[:], rhs=gstt[:],
                start=True, stop=True,
            )
            o2 = o1 + n_stt
            nc.tensor.matmul(
                out=psA[:, o2:o2 + NT], lhsT=ones_f[:], rhs=accQ[:],
                start=True, stop=True,
            )
            o3 = o2 + NT
            nc.tensor.matmul(
                out=psA[:, o3:o3 + n_actb], lhsT=ones_f[:],
                rhs=accB[:], start=True, stop=True,
            )
            o4 = o3 + n_actb

            # ACT combine; elementwise outputs land in act_scr so a
            # same-engine WAW keeps these after the whole main ACT stream
            # fA = -4 * (sum G + sum gstt)   (the -4*x_t correction)
            nc.scalar.activation(
                pin[0:1, 0:o2], psA[:, 0:o2], Act.Identity,
                bias=0.0, scale=-4.0, accum_out=fin[:, 0:1],
            )
            # fB = sum (c+1)^2 over all tiles
            nc.scalar.activation(
                pin[0:1, 0:NT], psA[:, o2:o3], Act.Identity,
                bias=0.0, scale=1.0, accum_out=fin[:, 1:2],
            )
            # fC = 4 * sum relu(x-1) over STT tiles
            nc.scalar.activation(
                pin[0:1, 0:n_actb], psA[:, o3:o4], Act.Identity,
                bias=0.0, scale=4.0, accum_out=fin[:, 2:3],
            )
            g1 = small.tile([1, 1], f32, tag="g1")
            nc.scalar.activation(
                g1[:], fin[:, 0:1], Act.Identity, bias=fin[:, 1:2], scale=1.0
            )
            g2 = small.tile([1, 1], f32, tag="g2")
            nc.scalar.activation(
                g2[:], fin[:, 2:3], Act.Identity, bias=fin[:, 3:4], scale=1.0
            )
            g3 = small.tile([1, 1], f32, tag="g3")
            nc.scalar.activation(
                g3[:], g1[:], Act.Identity, bias=g2[:], scale=1.0
            )
            # res = g3/N + const/N, const = -4*FD*P per DVE-B tile
            # (w = 4*max(x,1) = 4*relu(x-1) + 4 per element)
            biasc = (-4.0 * FD * P * n_dveb) / N_TOTAL
            bias_t = small.tile([1, 1], f32, tag="bias")
            nc.vector.memset(bias_t[:], biasc)
            res = small.tile([1, 1], f32, tag="res")
            nc.scalar.activation(
                res[:], g3[:], Act.Identity, bias=bias_t[:],
                scale=1.0 / N_TOTAL,
            )
            nc.sync.dma_start(out=out.ap(), in_=res[:])

    nc.compile()
    return nc


_NC_CACHE = None
LAST_RESULTS = None


def kernel(input, target):
    global _NC_CACHE, LAST_RESULTS
    x = np.ascontiguousarray(np.asarray(input, dtype=np.float32))
    tg = np.ascontiguousarray(np.asarray(target).astype(np.int64))
    assert x.shape == (N_TOTAL, C), x.shape
    assert tg.shape == (N_TOTAL,), tg.shape

    if _NC_CACHE is None:
        _NC_CACHE = build_program()
    nc = _NC_CACHE

    # flat element offset of each row's target within its core shard
    offs_all = (
        np.tile(np.arange(ROWS, dtype=np.int64) * C, N_CORES) + tg
    ).astype(np.int32)
    tc_all = tg.astype(np.float32)

    in_maps = [
        {
            "x": x[c * ROWS:(c + 1) * ROWS],
            "og": offs_all[c * ROWS:(c + 1) * ROWS],
            "tc": tc_all[c * ROWS:(c + 1) * ROWS],
        }
        for c in range(N_CORES)
    ]
    res = run_bass_kernel_spmd(nc, in_maps, core_ids=list(range(N_CORES)))
    LAST_RESULTS = res
    total = np.float32(0.0)
    for r in res.results:
        total += np.float32(r["out"].reshape(()))
    return np.asarray(total, dtype=np.float32)


if __name__ == "__main__":
    rng = np.random.default_rng(0)
    xs = rng.standard_normal((N_TOTAL, C), dtype=np.float32)
    ts = rng.integers(0, C, size=(N_TOTAL,)).astype(np.int64)
    got = kernel(xs, ts)
    m = np.where(np.arange(C)[None, :] == ts[:, None], xs, -xs)
    hinge = np.maximum(0.0, 1.0 - m)
    loss = np.where(m >= -1.0, hinge * hinge, -4.0 * m)
    want = loss.sum(dtype=np.float64) / N_TOTAL
    print("got", got, "want", want, "rel", abs(got - want) / abs(want))


# revision 19
# speedup vs baseline: 1.2986x; 1.0215x over previous
"""MultiHuberLoss Trainium2 kernel (v2).

Reference (per element, with m = +x at the target class, -x elsewhere):
    hinge = max(0, 1 - m);  loss = where(m >= -1, hinge^2, -4m);  out = sum(loss)/N

Identities (exact):
  F(m) = relu(1-m)^2 - relu(-1-m)^2
  Main pass uses m = -x for EVERY element:
      F(-x) = (clamp(x,-1,1)+1)^2 + 4*relu(x-1)
  Per-row correction for the target column t (where m = +x_t):
      F(x_t) - F(-x_t) = -4 * x_t

Engine split (per core, 16 tiles of [128, 4000] fp32):
  - DVE:   c = clamp(x,-1,1) -> fp16            (tensor_scalar, 2x_2P mode)
           w = 4*max(x,1)    -> fp16            (tensor_scalar, 2x_2P mode;
                                                 12 non-STT tiles)
  - ACT:   Square(c+1) with accum_out -> accQ   (all 16 tiles)
           Relu(x-1)  with accum_out -> accB    (4 STT tiles only)
  - PE:    per-chunk matmul(ones, w) accumulated into one PSUM bank --
           replaces the 1x-mode DVE accumulate pass of the baseline
  - target extraction: 48 row-groups via gpsimd indirect-DMA gathers,
           16 row-groups via DVE is_equal masks (STT tiles)
  - finale: entirely on PE (partition-sums of G/gstt/accQ/accB into PSUM)
           + ACT (PSUM reads, combine, scale); the DVE stream ends with the
           main loop, so the gather chain never head-of-line blocks it.
           ACT finale instructions write their (discarded) elementwise
           outputs into the act_scr scratch so a same-engine WAW pins them
           after the whole main ACT stream.
"""

import numpy as np

import concourse.bacc as bacc
import concourse.bass as bass
import concourse.mybir as mybir
from concourse.bass_utils import run_bass_kernel_spmd
from concourse.tile import TileContext

N_TOTAL = 65536
C = 1000
N_CORES = 8
ROWS = N_TOTAL // N_CORES  # 8192 rows per core
P = 128                    # partitions
JPP = ROWS // P            # 64 rows per partition
FREE = JPP * C             # 64000 f32 per partition
FD = 4000                  # free-dim tile size (4 whole rows per partition)
NT = FREE // FD            # 16 tiles
RPT = FD // C              # rows per partition per tile (4)
CH = 500                   # PE chunk width (fits one PSUM bank)

# tiles whose sub-rows get their target extracted via DVE is_equal-mask
# (28 row-groups; the other 36 row-groups use gpsimd indirect gathers)
STT_TILES = (1, 3, 5, 7, 9, 11, 13)
# tiles whose B-term (4*sum max(x,1)) runs on DVE+PE; the rest use ACT Relu
DVEB_TILES = (0, 2, 4, 6, 8, 10, 12, 13, 14, 15)

f32 = mybir.dt.float32
f16 = mybir.dt.float16
i32 = mybir.dt.int32
Alu = mybir.AluOpType
Act = mybir.ActivationFunctionType


def build_program():
    nc = bacc.Bacc(
        "TRN2", target_bir_lowering=False, debug=False, num_devices=N_CORES
    )
    x = nc.dram_tensor("x", [ROWS, C], f32, kind="ExternalInput")
    # host-precomputed flat element offsets: og[r] = r*C + target[r]
    og = nc.dram_tensor("og", [ROWS], i32, kind="ExternalInput")
    # target column of each row as f32 (for the is_equal extraction)
    tc_in = nc.dram_tensor("tc", [ROWS], f32, kind="ExternalInput")
    out = nc.dram_tensor("out", [1, 1], f32, kind="ExternalOutput")

    x_flat = x.ap().rearrange("(p j) c -> p (j c)", p=P)  # [128, 64000]
    x_lin = x.ap().rearrange("a (b one) -> (a b) one", one=1)  # [8192000, 1]
    og2d = og.ap().rearrange("(p j) -> p j", p=P)         # [128, 64]
    tc2d = tc_in.ap().rearrange("(p j) -> p j", p=P)      # [128, 64]

    stt_js = {t * RPT + j for t in STT_TILES for j in range(RPT)}
    n_stt = len(stt_js)
    n_dveb = len(DVEB_TILES)
    n_actb = NT - n_dveb

    with TileContext(nc) as tc:
        with (
            tc.tile_pool(name="xp", bufs=5) as xp,
            tc.tile_pool(name="xq", bufs=4) as xq,
            tc.tile_pool(name="cp", bufs=4) as cp,
            tc.tile_pool(name="cq", bufs=4) as cq,
            tc.tile_pool(name="wp", bufs=4) as wp,
            tc.tile_pool(name="wq", bufs=4) as wq,
            tc.tile_pool(name="scr", bufs=1) as scr,
            tc.tile_pool(name="small", bufs=1) as small,
            tc.tile_pool(name="psp", bufs=1, space="PSUM") as psp,
        ):
            ones_h = small.tile([P, 1], f16, tag="ones_h")
            nc.vector.memset(ones_h[:], 1.0)
            ones_f = small.tile([P, 1], f32, tag="ones_f")
            nc.vector.memset(ones_f[:], 1.0)
            negones = small.tile([P, 1], f32, tag="negones")
            nc.vector.memset(negones[:], -1.0)
            # column-index ramp 0..999, same on every partition (f32 exact)
            ci = small.tile([P, C], f32, tag="ci")
            nc.gpsimd.iota(
                ci[:], pattern=[[1, C]], base=0, channel_multiplier=0,
                allow_small_or_imprecise_dtypes=True,
            )

            # ---- gather path, traced FIRST so the offsets DMA leads the
            # Sync queue and the gathers start on gpsimd early ----
            offs = small.tile([P, JPP], i32, tag="offs")
            nc.sync.dma_start(out=offs[:], in_=og2d)
            tcv = small.tile([P, JPP], f32, tag="tcv")
            nc.sync.dma_start(out=tcv[:], in_=tc2d)
            # memset on gpsimd: keeps the gather prologue free of
            # cross-engine waits so the indirect chain starts early
            G = small.tile([P, JPP], f32, tag="G")
            nc.gpsimd.memset(G[:], 0.0)
            for j in range(JPP):
                if j in stt_js:
                    continue
                nc.gpsimd.indirect_dma_start(
                    out=G[:, j:j + 1],
                    out_offset=None,
                    in_=x_lin,
                    in_offset=bass.IndirectOffsetOnAxis(
                        ap=offs[:, j:j + 1], axis=0
                    ),
                )

            # ---- main streaming loop ----
            # tiles 0 and 15 run in 4 quarter-pieces (separate small tiles so
            # each piece's compute starts as soon as its own DMA lands):
            # cuts the pipeline fill at the start and the drain at the end
            EDGE_TILES = (0, NT - 1)
            n_qcols = NT - len(EDGE_TILES) + 4 * len(EDGE_TILES)
            accQ = small.tile([P, n_qcols], f32, tag="accQ")
            accB = small.tile([P, n_actb], f32, tag="accB")
            gstt = small.tile([P, max(1, n_stt)], f32, tag="gstt")
            psA = psp.tile([1, CH], f32, tag="psA")
            si = 0
            bi = 0
            qi = 0
            n_chunks = n_dveb * (FD // CH)
            ck = 0
            for t in range(NT):
                nq = 4 if t in EDGE_TILES else 1
                pd = FD // nq     # piece free-dim
                for q in range(nq):
                    col0 = t * FD + q * pd
                    if nq == 1:
                        xt = xp.tile([P, FD], f32)
                        h = FD // 2
                        nc.sync.dma_start(
                            out=xt[:, 0:h], in_=x_flat[:, col0:col0 + h]
                        )
                        nc.sync.dma_start(
                            out=xt[:, h:FD], in_=x_flat[:, col0 + h:col0 + FD]
                        )
                    else:
                        xt = xq.tile([P, pd], f32, tag="xq")
                        nc.sync.dma_start(
                            out=xt[:], in_=x_flat[:, col0:col0 + pd]
                        )
                    if nq == 4:
                        c = cq.tile([P, pd], f16, tag="cq")
                    else:
                        c = cp.tile([P, FD], f16, tag="cp")
                    nc.vector.tensor_scalar(
                        c[:], xt[:], -1.0, 1.0, Alu.max, Alu.min
                    )
                    # accQ col = sum (c+1)^2
                    sq = scr.tile([P, FD], f16, tag="act_scr")
                    nc.scalar.activation(
                        sq[0:P, 0:pd],
                        c[:],
                        Act.Square,
                        bias=1.0,
                        scale=1.0,
                        accum_out=accQ[:, qi:qi + 1],
                    )
                    qi += 1
                    if t in DVEB_TILES:
                        # w = 4*max(x,1) (fp16), PE-summed into psA
                        if nq == 4:
                            w = wq.tile([P, pd], f16, tag="wq")
                        else:
                            w = wp.tile([P, FD], f16, tag="wp")
                        nc.vector.tensor_scalar(
                            w[:], xt[:], 1.0, 4.0, Alu.max, Alu.mult
                        )
                        for k in range(pd // CH):
                            nc.tensor.matmul(
                                out=psA[:],
                                lhsT=ones_h[:],
                                rhs=w[:, k * CH:(k + 1) * CH],
                                start=(ck == 0),
                                stop=(ck == n_chunks - 1),
                            )
                            ck += 1
                    else:
                        # accB col = sum relu(x-1) (B-term on ACT)
                        rl = scr.tile([P, FD], f16, tag="act_scr")
                        nc.scalar.activation(
                            rl[0:P, 0:pd],
                            xt[:],
                            Act.Relu,
                            bias=negones[:],
                            scale=1.0,
                            accum_out=accB[:, bi:bi + 1],
                        )
                        if q == nq - 1:
                            bi += 1
                    if t in STT_TILES:
                        # per-sub-row target extraction:
                        #   gstt col = sum( (ci == target_col) * x_subrow )
                        for j in (range(RPT) if nq == 1 else (q,)):
                            jg = t * RPT + j
                            xin = (xt[:, (j - q * RPT) * C:
                                      (j - q * RPT + 1) * C] if nq == 1
                                   else xt[:, 0:C])
                            ws = scr.tile([P, C], f32, tag="w_stt")
                            nc.vector.scalar_tensor_tensor(
                                out=ws[:],
                                in0=ci[:],
                                scalar=tcv[:, jg:jg + 1],
                                in1=xin,
                                op0=Alu.is_equal,
                                op1=Alu.mult,
                                accum_out=gstt[:, si:si + 1],
                            )
                            si += 1

            # ---- finale ----
            # fD = sum 4*max(x,1) over the 12 DVE-B tiles, read out FIRST
            # (the PE partition-sums below clobber psA columns; reusing the
            # psA tile gives the finale matmuls a WAW dep on every chunk
            # matmul, so the scheduler cannot hoist them ahead of the main
            # loop and head-of-line block the PE queue on the gather chain)
            fin = small.tile([1, 8], f32, tag="fin")
            pin = scr.tile([P, FD], f16, tag="act_scr")
            nc.scalar.activation(
                pin[0:1, 0:CH], psA[:], Act.Identity,
                bias=0.0, scale=1.0, accum_out=fin[:, 3:4],
            )
            # PE partition-sums of the small accumulators into psA columns
            nc.tensor.matmul(
                out=psA[:, 0:JPP], lhsT=ones_f[:], rhs=G[:],
                start=True, stop=True,
            )
            o1 = JPP
            nc.tensor.matmul(
                out=psA[:, o1:o1 + n_stt], lhsT=ones_f[:], rhs=gstt[:],
                start=True, stop=True,
            )
            o2 = o1 + n_stt
            nc.tensor.matmul(
                out=psA[:, o2:o2 + n_qcols], lhsT=ones_f[:], rhs=accQ[:],
                start=True, stop=True,
            )
            o3 = o2 + n_qcols
            nc.tensor.matmul(
                out=psA[:, o3:o3 + n_actb], lhsT=ones_f[:],
                rhs=accB[:], start=True, stop=True,
            )
            o4 = o3 + n_actb

            # ACT combine; elementwise outputs land in act_scr so a
            # same-engine WAW keeps these after the whole main ACT stream
            # fA = -4 * (sum G + sum gstt)   (the -4*x_t correction)
            nc.scalar.activation(
                pin[0:1, 0:o2], psA[:, 0:o2], Act.Identity,
                bias=0.0, scale=-4.0, accum_out=fin[:, 0:1],
            )
            # fB = sum (c+1)^2 over all tiles
            nc.scalar.activation(
                pin[0:1, 0:n_qcols], psA[:, o2:o3], Act.Identity,
                bias=0.0, scale=1.0, accum_out=fin[:, 1:2],
            )
            # fC = 4 * sum relu(x-1) over STT tiles
            nc.scalar.activation(
                pin[0:1, 0:n_actb], psA[:, o3:o4], Act.Identity,
                bias=0.0, scale=4.0, accum_out=fin[:, 2:3],
            )
            g1 = small.tile([1, 1], f32, tag="g1")
            nc.scalar.activation(
                g1[:], fin[:, 0:1], Act.Identity, bias=fin[:, 1:2], scale=1.0
            )
            g2 = small.tile([1, 1], f32, tag="g2")
            nc.scalar.activation(
                g2[:], fin[:, 2:3], Act.Identity, bias=fin[:, 3:4], scale=1.0
            )
            g3 = small.tile([1, 1], f32, tag="g3")
            nc.scalar.activation(
                g3[:], g1[:], Act.Identity, bias=g2[:], scale=1.0
            )
            # res = g3/N + const/N, const = -4*FD*P per DVE-B tile
            # (w = 4*max(x,1) = 4*relu(x-1) + 4 per element)
            biasc = (-4.0 * FD * P * n_dveb) / N_TOTAL
            bias_t = small.tile([1, 1], f32, tag="bias")
            nc.vector.memset(bias_t[:], biasc)
            res = small.tile([1, 1], f32, tag="res")
            nc.scalar.activation(
                res[:], g3[:], Act.Identity, bias=bias_t[:],
                scale=1.0 / N_TOTAL,
            )
            nc.sync.dma_start(out=out.ap(), in_=res[:])

    nc.compile()
    return nc


_NC_CACHE = None
LAST_RESULTS = None


def kernel(input, target):
    global _NC_CACHE, LAST_RESULTS
    x = np.ascontiguousarray(np.asarray(input, dtype=np.float32))
    tg = np.ascontiguousarray(np.asarray(target).astype(np.int64))
    assert x.shape == (N_TOTAL, C), x.shape
    assert tg.shape == (N_TOTAL,), tg.shape

    if _NC_CACHE is None:
        _NC_CACHE = build_program()
    nc = _NC_CACHE

    # flat element offset of each row's target within its core shard
    offs_all = (
        np.tile(np.arange(ROWS, dtype=np.int64) * C, N_CORES) + tg
    ).astype(np.int32)
    tc_all = tg.astype(np.float32)

    in_maps = [
        {
            "x": x[c * ROWS:(c + 1) * ROWS],
            "og": offs_all[c * ROWS:(c + 1) * ROWS],
            "tc": tc_all[c * ROWS:(c + 1) * ROWS],
        }
        for c in range(N_CORES)
    ]
    res = run_bass_kernel_spmd(nc, in_maps, core_ids=list(range(N_CORES)))
    LAST_RESULTS = res
    total = np.float32(0.0)
    for r in res.results:
        total += np.float32(r["out"].reshape(()))
    return np.asarray(total, dtype=np.float32)


if __name__ == "__main__":
    rng = np.random.default_rng(0)
    xs = rng.standard_normal((N_TOTAL, C), dtype=np.float32)
    ts = rng.integers(0, C, size=(N_TOTAL,)).astype(np.int64)
    got = kernel(xs, ts)
    m = np.where(np.arange(C)[None, :] == ts[:, None], xs, -xs)
    hinge = np.maximum(0.0, 1.0 - m)
    loss = np.where(m >= -1.0, hinge * hinge, -4.0 * m)
    want = loss.sum(dtype=np.float64) / N_TOTAL
    print("got", got, "want", want, "rel", abs(got - want) / abs(want))


# revision 22
# speedup vs baseline: 1.3119x; 1.0102x over previous
"""MultiHuberLoss Trainium2 kernel (v2).

Reference (per element, with m = +x at the target class, -x elsewhere):
    hinge = max(0, 1 - m);  loss = where(m >= -1, hinge^2, -4m);  out = sum(loss)/N

Identities (exact):
  F(m) = relu(1-m)^2 - relu(-1-m)^2
  Main pass uses m = -x for EVERY element:
      F(-x) = (clamp(x,-1,1)+1)^2 + 4*relu(x-1)
  Per-row correction for the target column t (where m = +x_t):
      F(x_t) - F(-x_t) = -4 * x_t

Engine split (per core, 16 tiles of [128, 4000] fp32):
  - DVE:   c = clamp(x,-1,1) -> fp16            (tensor_scalar, 2x_2P mode)
           w = 4*max(x,1)    -> fp16            (tensor_scalar, 2x_2P mode;
                                                 12 non-STT tiles)
  - ACT:   Square(c+1) with accum_out -> accQ   (all 16 tiles)
           Relu(x-1)  with accum_out -> accB    (4 STT tiles only)
  - PE:    per-chunk matmul(ones, w) accumulated into one PSUM bank --
           replaces the 1x-mode DVE accumulate pass of the baseline
  - target extraction: 48 row-groups via gpsimd indirect-DMA gathers,
           16 row-groups via DVE is_equal masks (STT tiles)
  - finale: entirely on PE (partition-sums of G/gstt/accQ/accB into PSUM)
           + ACT (PSUM reads, combine, scale); the DVE stream ends with the
           main loop, so the gather chain never head-of-line blocks it.
           ACT finale instructions write their (discarded) elementwise
           outputs into the act_scr scratch so a same-engine WAW pins them
           after the whole main ACT stream.
"""

import numpy as np

import concourse.bacc as bacc
import concourse.bass as bass
import concourse.mybir as mybir
from concourse.bass_utils import run_bass_kernel_spmd
from concourse.tile import TileContext

N_TOTAL = 65536
C = 1000
N_CORES = 8
ROWS = N_TOTAL // N_CORES  # 8192 rows per core
P = 128                    # partitions
JPP = ROWS // P            # 64 rows per partition
FREE = JPP * C             # 64000 f32 per partition
FD = 4000                  # free-dim tile size (4 whole rows per partition)
NT = FREE // FD            # 16 tiles
RPT = FD // C              # rows per partition per tile (4)
CH = 500                   # PE chunk width (fits one PSUM bank)

# tiles whose sub-rows get their target extracted via DVE is_equal-mask
# (24 row-groups; the other 40 row-groups use gpsimd indirect gathers)
STT_TILES = (1, 3, 5, 7, 9, 11)
# tiles whose B-term (4*sum max(x,1)) runs on DVE+PE; the rest use ACT Relu
DVEB_TILES = (0, 2, 4, 6, 8, 10, 12, 13, 14, 15)

f32 = mybir.dt.float32
f16 = mybir.dt.float16
i32 = mybir.dt.int32
Alu = mybir.AluOpType
Act = mybir.ActivationFunctionType


def build_program():
    nc = bacc.Bacc(
        "TRN2", target_bir_lowering=False, debug=False, num_devices=N_CORES
    )
    x = nc.dram_tensor("x", [ROWS, C], f32, kind="ExternalInput")
    # host-precomputed flat element offsets: og[r] = r*C + target[r]
    og = nc.dram_tensor("og", [ROWS], i32, kind="ExternalInput")
    # target column of each row as f32 (for the is_equal extraction)
    tc_in = nc.dram_tensor("tc", [ROWS], f32, kind="ExternalInput")
    out = nc.dram_tensor("out", [1, 1], f32, kind="ExternalOutput")

    x_flat = x.ap().rearrange("(p j) c -> p (j c)", p=P)  # [128, 64000]
    x_lin = x.ap().rearrange("a (b one) -> (a b) one", one=1)  # [8192000, 1]
    og2d = og.ap().rearrange("(p j) -> p j", p=P)         # [128, 64]
    tc2d = tc_in.ap().rearrange("(p j) -> p j", p=P)      # [128, 64]

    stt_js = {t * RPT + j for t in STT_TILES for j in range(RPT)}
    n_stt = len(stt_js)
    n_dveb = len(DVEB_TILES)
    n_actb = NT - n_dveb

    with TileContext(nc) as tc:
        with (
            tc.tile_pool(name="xp", bufs=5) as xp,
            tc.tile_pool(name="xq", bufs=4) as xq,
            tc.tile_pool(name="cp", bufs=4) as cp,
            tc.tile_pool(name="cq", bufs=4) as cq,
            tc.tile_pool(name="wp", bufs=4) as wp,
            tc.tile_pool(name="wq", bufs=4) as wq,
            tc.tile_pool(name="scr", bufs=1) as scr,
            tc.tile_pool(name="small", bufs=1) as small,
            tc.tile_pool(name="psp", bufs=1, space="PSUM") as psp,
        ):
            ones_h = small.tile([P, 1], f16, tag="ones_h")
            nc.vector.memset(ones_h[:], 1.0)
            ones_f = small.tile([P, 1], f32, tag="ones_f")
            nc.vector.memset(ones_f[:], 1.0)
            m4ones_f = small.tile([P, 1], f32, tag="m4ones_f")
            nc.vector.memset(m4ones_f[:], -4.0)
            p4ones_f = small.tile([P, 1], f32, tag="p4ones_f")
            nc.vector.memset(p4ones_f[:], 4.0)
            negones = small.tile([P, 1], f32, tag="negones")
            nc.vector.memset(negones[:], -1.0)
            # column-index ramp 0..999, same on every partition (f32 exact)
            ci = small.tile([P, C], f32, tag="ci")
            nc.gpsimd.iota(
                ci[:], pattern=[[1, C]], base=0, channel_multiplier=0,
                allow_small_or_imprecise_dtypes=True,
            )

            # ---- gather path, traced FIRST so the offsets DMA leads the
            # Sync queue and the gathers start on gpsimd early ----
            offs = small.tile([P, JPP], i32, tag="offs")
            nc.sync.dma_start(out=offs[:], in_=og2d)
            tcv = small.tile([P, JPP], f32, tag="tcv")
            nc.sync.dma_start(out=tcv[:], in_=tc2d)
            # memset on gpsimd: keeps the gather prologue free of
            # cross-engine waits so the indirect chain starts early
            G = small.tile([P, JPP], f32, tag="G")
            nc.gpsimd.memset(G[:], 0.0)
            for j in range(JPP):
                if j in stt_js:
                    continue
                nc.gpsimd.indirect_dma_start(
                    out=G[:, j:j + 1],
                    out_offset=None,
                    in_=x_lin,
                    in_offset=bass.IndirectOffsetOnAxis(
                        ap=offs[:, j:j + 1], axis=0
                    ),
                )

            # ---- main streaming loop ----
            # tiles 0 and 15 run in 4 quarter-pieces (separate small tiles so
            # each piece's compute starts as soon as its own DMA lands):
            # cuts the pipeline fill at the start and the drain at the end
            EDGE_TILES = (0, NT - 1)
            n_qcols = NT - len(EDGE_TILES) + 4 * len(EDGE_TILES)
            accQ = small.tile([P, n_qcols], f32, tag="accQ")
            accB = small.tile([P, n_actb], f32, tag="accB")
            gstt = small.tile([P, max(1, n_stt)], f32, tag="gstt")
            psA = psp.tile([1, CH], f32, tag="psA")
            si = 0
            bi = 0
            qi = 0
            n_chunks = n_dveb * (FD // CH)
            ck = 0
            for t in range(NT):
                nq = 4 if t in EDGE_TILES else 1
                pd = FD // nq     # piece free-dim
                for q in range(nq):
                    col0 = t * FD + q * pd
                    if nq == 1:
                        xt = xp.tile([P, FD], f32)
                        h = FD // 2
                        nc.sync.dma_start(
                            out=xt[:, 0:h], in_=x_flat[:, col0:col0 + h]
                        )
                        nc.sync.dma_start(
                            out=xt[:, h:FD], in_=x_flat[:, col0 + h:col0 + FD]
                        )
                    else:
                        xt = xq.tile([P, pd], f32, tag="xq")
                        nc.sync.dma_start(
                            out=xt[:], in_=x_flat[:, col0:col0 + pd]
                        )
                    if nq == 4:
                        c = cq.tile([P, pd], f16, tag="cq")
                    else:
                        c = cp.tile([P, FD], f16, tag="cp")
                    nc.vector.tensor_scalar(
                        c[:], xt[:], -1.0, 1.0, Alu.max, Alu.min
                    )
                    # accQ col = sum (c+1)^2
                    sq = scr.tile([P, FD], f16, tag="act_scr")
                    nc.scalar.activation(
                        sq[0:P, 0:pd],
                        c[:],
                        Act.Square,
                        bias=1.0,
                        scale=1.0,
                        accum_out=accQ[:, qi:qi + 1],
                    )
                    qi += 1
                    if t in DVEB_TILES:
                        # w = 4*max(x,1) (fp16), PE-summed into psA
                        if nq == 4:
                            w = wq.tile([P, pd], f16, tag="wq")
                        else:
                            w = wp.tile([P, FD], f16, tag="wp")
                        nc.vector.tensor_scalar(
                            w[:], xt[:], 1.0, 4.0, Alu.max, Alu.mult
                        )
                        for k in range(pd // CH):
                            nc.tensor.matmul(
                                out=psA[:],
                                lhsT=ones_h[:],
                                rhs=w[:, k * CH:(k + 1) * CH],
                                start=(ck == 0),
                                stop=(ck == n_chunks - 1),
                            )
                            ck += 1
                    else:
                        # accB col = sum relu(x-1) (B-term on ACT)
                        rl = scr.tile([P, FD], f16, tag="act_scr")
                        nc.scalar.activation(
                            rl[0:P, 0:pd],
                            xt[:],
                            Act.Relu,
                            bias=negones[:],
                            scale=1.0,
                            accum_out=accB[:, bi:bi + 1],
                        )
                        if q == nq - 1:
                            bi += 1
                    if t in STT_TILES:
                        # per-sub-row target extraction:
                        #   gstt col = sum( (ci == target_col) * x_subrow )
                        for j in (range(RPT) if nq == 1 else (q,)):
                            jg = t * RPT + j
                            xin = (xt[:, (j - q * RPT) * C:
                                      (j - q * RPT + 1) * C] if nq == 1
                                   else xt[:, 0:C])
                            ws = scr.tile([P, C], f32, tag="w_stt")
                            nc.vector.scalar_tensor_tensor(
                                out=ws[:],
                                in0=ci[:],
                                scalar=tcv[:, jg:jg + 1],
                                in1=xin,
                                op0=Alu.is_equal,
                                op1=Alu.mult,
                                accum_out=gstt[:, si:si + 1],
                            )
                            si += 1

            # ---- finale ----
            # fin[2] = the constant term: -4*FD*P per DVE-B tile
            # (w = 4*max(x,1) = 4*relu(x-1) + 4 per element)
            fin = small.tile([1, 8], f32, tag="fin")
            nc.vector.memset(fin[:, 2:3], -4.0 * FD * P * n_dveb)
            # fin[1] = sum 4*max(x,1) over the DVE-B tiles, read out FIRST
            # (the PE partition-sums below clobber psA columns; reusing the
            # psA tile gives the finale matmuls a WAW dep on every chunk
            # matmul, so the scheduler cannot hoist them ahead of the main
            # loop and head-of-line block the PE queue on the gather chain)
            pin = scr.tile([P, FD], f16, tag="act_scr")
            nc.scalar.activation(
                pin[0:1, 0:CH], psA[:], Act.Identity,
                bias=0.0, scale=1.0, accum_out=fin[:, 1:2],
            )
            # PE partition-sums of the small accumulators into psA columns,
            # pre-scaled by the lhsT weights so one ACT read combines them:
            #   -4*(sum G + sum gstt) + sum (c+1)^2 + 4*sum relu(x-1)
            nc.tensor.matmul(
                out=psA[:, 0:JPP], lhsT=m4ones_f[:], rhs=G[:],
                start=True, stop=True,
            )
            o1 = JPP
            nc.tensor.matmul(
                out=psA[:, o1:o1 + n_stt], lhsT=m4ones_f[:], rhs=gstt[:],
                start=True, stop=True,
            )
            o2 = o1 + n_stt
            nc.tensor.matmul(
                out=psA[:, o2:o2 + n_qcols], lhsT=ones_f[:], rhs=accQ[:],
                start=True, stop=True,
            )
            o3 = o2 + n_qcols
            nc.tensor.matmul(
                out=psA[:, o3:o3 + n_actb], lhsT=p4ones_f[:],
                rhs=accB[:], start=True, stop=True,
            )
            o4 = o3 + n_actb
            nc.scalar.activation(
                pin[0:1, 0:o4], psA[:, 0:o4], Act.Identity,
                bias=0.0, scale=1.0, accum_out=fin[:, 0:1],
            )
            # res = (fin[0] + fin[1] + fin[2]) / N
            res = small.tile([1, 1], f32, tag="res")
            nc.scalar.activation(
                pin[0:1, 0:3], fin[:, 0:3], Act.Identity,
                bias=0.0, scale=1.0 / N_TOTAL, accum_out=res[:],
            )
            nc.sync.dma_start(out=out.ap(), in_=res[:])

    nc.compile()
    return nc


_NC_CACHE = None
LAST_RESULTS = None


def kernel(input, target):
    global _NC_CACHE, LAST_RESULTS
    x = np.ascontiguousarray(np.asarray(input, dtype=np.float32))
    tg = np.ascontiguousarray(np.asarray(target).astype(np.int64))
    assert x.shape == (N_TOTAL, C), x.shape
    assert tg.shape == (N_TOTAL,), tg.shape

    if _NC_CACHE is None:
        _NC_CACHE = build_program()
    nc = _NC_CACHE

    # flat element offset of each row's target within its core shard
    offs_all = (
        np.tile(np.arange(ROWS, dtype=np.int64) * C, N_CORES) + tg
    ).astype(np.int32)
    tc_all = tg.astype(np.float32)

    in_maps = [
        {
            "x": x[c * ROWS:(c + 1) * ROWS],
            "og": offs_all[c * ROWS:(c + 1) * ROWS],
            "tc": tc_all[c * ROWS:(c + 1) * ROWS],
        }
        for c in range(N_CORES)
    ]
    res = run_bass_kernel_spmd(nc, in_maps, core_ids=list(range(N_CORES)))
    LAST_RESULTS = res
    total = np.float32(0.0)
    for r in res.results:
        total += np.float32(r["out"].reshape(()))
    return np.asarray(total, dtype=np.float32)


if __name__ == "__main__":
    rng = np.random.default_rng(0)
    xs = rng.standard_normal((N_TOTAL, C), dtype=np.float32)
    ts = rng.integers(0, C, size=(N_TOTAL,)).astype(np.int64)
    got = kernel(xs, ts)
    m = np.where(np.arange(C)[None, :] == ts[:, None], xs, -xs)
    hinge = np.maximum(0.0, 1.0 - m)
    loss = np.where(m >= -1.0, hinge * hinge, -4.0 * m)
    want = loss.sum(dtype=np.float64) / N_TOTAL
    print("got", got, "want", want, "rel", abs(got - want) / abs(want))
